# revision 1
# baseline (speedup 1.0000x reference)
"""Trainium2 Bass kernel for MembraneNet (PINN forward + analytic PDE residual).

Math: the reference computes, per collocation point p=(x,y):
  u(p)  = Wout . h3 + bout           (4-layer tanh MLP, H=64)
  PDE   = K*(uxx+uyy) + Kx*ux + Ky*uy + f
The reference builds per-point (H,H) Jacobian chain products; here we use
forward-mode propagation of (h, dh/dx, dh/dy, lap h) per layer which is
O(H^2) per point instead of O(H^3):
  z  = W h + b ;  h' = tanh(z) ;  d = 1-h'^2 ;  s = -2 h' d
  zx = W gx    ;  gx' = d . zx
  zy = W gy    ;  gy' = d . zy
  zl = W lap   ;  lap' = s . (zx^2 + zy^2) + d . zl
Then u = Wout.h3+bout, ux = Wout.gx3, uy = Wout.gy3, uxx+uyy = Wout.lap3.

Sharding: batch (16384) split over 8 cores (2048/core). On each core the 2048
points are stacked as 2 chunks of 1024 on SBUF partitions 0-63 / 64-127 with
block-diagonal replicated weights, so matmuls use the full 128-wide PE array.
gx is propagated with a flipped sign (one fewer op at layer 0); squares are
unaffected and the final PDE assembly subtracts the Kx/Ky terms instead.
lap is carried unsummed as [dd, ee]; the next layer's zl matmul accumulates
both in PSUM, trading a cheap extra PE pass for a DVE add on the critical path.
"""

import sys

sys.path.insert(0, "/opt/trn_rl_repo")

import numpy as np
from contextlib import ExitStack

import concourse.bass as bass
import concourse.mybir as mybir
import concourse.tile as tile
from concourse.masks import make_identity

B = 16384
H = 64
L = 4
NCORES = 8
BC = B // NCORES          # 2048 points per core
F = BC // 2               # 1024 free-dim columns (2 chunks stacked)
NH = 512                  # matmul free-dim per instruction (1 PSUM bank fp32)
NMM = F // NH             # matmul instructions per pass
FT = BC // 128            # 16: free dim of final per-point [128, FT] tiles

f32 = mybir.dt.float32
AF = mybir.ActivationFunctionType
OP = mybir.AluOpType

# dtype knobs: matmul operand dtype and elementwise dtype
MM_DT = mybir.dt.float32r  # f32r: 1 cyc/row on PE vs 4 for f32
EW_DT = f32


def _col(ap):
    """[64] dram vector -> [64,1] view for column DMA."""
    return ap[:, None]


def _patch_walrus_ldw_opt():
    """Re-enable walrus LDWEIGHTS dedup (concourse pins it off)."""
    from concourse import bass_utils as _bu

    if getattr(_bu, "_ldw_opt_patched", False):
        return
    orig = _bu.run_command

    def patched(argv, **kw):
        argv = [
            "--enable-ldw-opt=true" if a == "--enable-ldw-opt=false" else a
            for a in argv
        ]
        return orig(argv, **kw)

    _bu.run_command = patched
    _bu._ldw_opt_patched = True


def _legalize_sync_waits(bj: bytes) -> bytes:
    """The walrus in this container accepts at most ONE on_wait per
    instruction, but Tile emits several. Move excess waits into standalone
    EventSemaphore instructions right before the owner (same engine, so the
    sequencer executes them first) — the exact encoding raw-bass wait_ge uses.
    """
    import json

    m = json.loads(bj)
    n = 0
    for fn in m.get("functions", []):
        for blk in fn.get("blocks", []):
            out = []
            for ins in blk.get("instructions", []):
                si = ins.get("sync_info") or {}
                waits = si.get("on_wait") or []
                if len(waits) > 1:
                    for w in waits[:-1]:
                        n += 1
                        out.append(
                            {
                                "name": f"lsw_{n}",
                                "opcode": "EventSemaphore",
                                "engine": ins["engine"],
                                "ins": [],
                                "outs": [],
                                "debug": ins.get("debug", 0),
                                "sync_info": {"on_update": [], "on_wait": [w]},
                            }
                        )
                    si["on_wait"] = waits[-1:]
                out.append(ins)
            blk["instructions"] = out
    return json.dumps(m).encode()


def build_nc(mm_dt=MM_DT, ew_dt=EW_DT):
    nc = bass.Bass()

    # ---- I/O ----
    xy_d = nc.dram_tensor("xy", [BC, 2], f32, kind="ExternalInput")
    K_d = nc.dram_tensor("K", [BC], f32, kind="ExternalInput")
    Kx_d = nc.dram_tensor("Kx", [BC], f32, kind="ExternalInput")
    Ky_d = nc.dram_tensor("Ky", [BC], f32, kind="ExternalInput")
    f_d = nc.dram_tensor("f", [BC], f32, kind="ExternalInput")
    W_d = [nc.dram_tensor("W0", [H, 2], f32, kind="ExternalInput")]
    b_d = [nc.dram_tensor("b0", [H], f32, kind="ExternalInput")]
    for i in range(1, L):
        W_d.append(nc.dram_tensor(f"W{i}", [H, H], f32, kind="ExternalInput"))
        b_d.append(nc.dram_tensor(f"b{i}", [H], f32, kind="ExternalInput"))
    Wout_d = nc.dram_tensor("Wout", [H], f32, kind="ExternalInput")
    bout_d = nc.dram_tensor("bout", [1], f32, kind="ExternalInput")
    u_d = nc.dram_tensor("u", [BC], f32, kind="ExternalOutput")
    pde_d = nc.dram_tensor("pde", [BC], f32, kind="ExternalOutput")

    with tile.TileContext(nc) as tc, ExitStack() as ctx:
        const = ctx.enter_context(tc.tile_pool(name="const", bufs=1))
        sb = ctx.enter_context(tc.tile_pool(name="sb", bufs=2))
        ps = ctx.enter_context(tc.tile_pool(name="ps", bufs=1, space="PSUM"))

        # ---- early DMAs (contiguous only; strided patterns are handled
        # on-chip via PE transpose / strided engine copies) ----
        # xy pairs, one partition per chunk
        xystg = sb.tile([128, 2 * F], f32, tag="xystg")
        nc.sync.dma_start(out=xystg[0:1, :], in_=xy_d[0:F, :])
        nc.scalar.dma_start(out=xystg[H : H + 1, :], in_=xy_d[F:BC, :])

        # W0 for both chunks (contiguous [64,2])
        w0c = const.tile([128, 2], f32, tag="w0c")
        nc.sync.dma_start(out=w0c[0:H, :], in_=W_d[0][:, :])
        nc.scalar.dma_start(out=w0c[H:128, :], in_=W_d[0][:, :])

        # bias/Wout row staging for the column transpose: row k = [b_k|b_k],
        # row 4 = [Wout|Wout]
        brows = sb.tile([128, 128], f32, tag="brows")
        for k in range(L):
            nc.sync.dma_start(out=brows[k : k + 1, 0:H], in_=b_d[k][None, :])
            nc.scalar.dma_start(out=brows[k : k + 1, H:128], in_=b_d[k][None, :])
        nc.sync.dma_start(out=brows[4:5, 0:H], in_=Wout_d[None, :])
        nc.scalar.dma_start(out=brows[4:5, H:128], in_=Wout_d[None, :])

        ident = const.tile([128, 128], f32, tag="ident")
        make_identity(nc, ident[:])

        # shared f32 zeros (memset cannot target f32r tiles)
        zF = const.tile([128, F], f32, tag="zF")
        nc.gpsimd.memset(zF[:], 0.0)

        # ---- weight prep: natural loads + PE transpose ----
        # block-diag(W, W) staged naturally, PE-transposed to block-diag(WT,WT).
        # Layer 0 (K=2): x/y input rows must sit on 32-aligned partitions, so
        # W0's x column goes to staging col 0 / y to col 32 (chunk B: 64/96),
        # making the transposed lhsT contract rhs partitions {0,32,64,96}.
        WT = []
        for k in range(L):
            wnat = sb.tile([128, 128], f32, tag="wnat")
            nc.gpsimd.memset(wnat[:], 0.0)
            if k == 0:
                nc.vector.tensor_copy(wnat[0:H, 0:1], w0c[0:H, 0:1])
                nc.vector.tensor_copy(wnat[0:H, 32:33], w0c[0:H, 1:2])
                nc.vector.tensor_copy(wnat[H:128, H : H + 1], w0c[H:128, 0:1])
                nc.vector.tensor_copy(
                    wnat[H:128, H + 32 : H + 33], w0c[H:128, 1:2]
                )
            else:
                nc.sync.dma_start(out=wnat[0:H, 0:H], in_=W_d[k][:, :])
                nc.scalar.dma_start(out=wnat[H:128, H:128], in_=W_d[k][:, :])
            wt_ps = ps.tile([128, 128], f32, tag="zlp")
            nc.tensor.transpose(wt_ps[:], wnat[:], ident[:])
            wt = const.tile([128, 128], mm_dt, tag=f"wt{k}")
            nc.scalar.copy(wt[:], wt_ps[:])
            WT.append(wt)

        # bias/Wout columns via one transpose: col k = [b_k;b_k], col4 = Wout
        bw_ps = ps.tile([128, 128], f32, tag="zyp")
        nc.tensor.transpose(bw_ps[:], brows[:], ident[:])
        bwc = const.tile([128, 8], f32, tag="bwc")
        nc.vector.tensor_copy(bwc[:], bw_ps[:, 0:8])
        bcol = [bwc[:, k : k + 1] for k in range(L)]
        wcol = bwc[:, 4:5]

        # output reduction lhsTs: one [128, 32] tile; 8-col group q has
        # local col 2q = [Wout;0], 2q+1 = [0;Wout]
        wl32f = sb.tile([128, 32], f32, tag="wl32f")
        nc.vector.memset(wl32f[:], 0.0)
        for q in range(4):
            nc.vector.tensor_copy(wl32f[0:H, 10 * q : 10 * q + 1], wcol[0:H])
            nc.vector.tensor_copy(
                wl32f[H:128, 10 * q + 1 : 10 * q + 2], wcol[H:128]
            )
        wl32 = const.tile([128, 32], mm_dt, tag="wl32")
        nc.vector.tensor_copy(wl32[:], wl32f[:])
        wout_l = [wl32[:, 8 * q : 8 * q + 8] for q in range(4)]

        # q0p1 = W0x^2 + W0y^2 per partition (layer-0 laplacian source)
        w0sq = const.tile([128, 2], f32, tag="w0sq")
        nc.vector.tensor_mul(w0sq[:], w0c[:], w0c[:])
        q0p1 = const.tile([128, 1], f32, tag="q0p1")
        nc.vector.tensor_reduce(
            out=q0p1[:], in_=w0sq[:], op=OP.add, axis=mybir.AxisListType.X
        )

        # bout broadcast column
        bout_c = const.tile([128, 1], f32, tag="bout_c")
        bout_bcast = bass.AP(
            tensor=bout_d[:].tensor, offset=0, ap=[[0, 128], [0, 1]]
        )
        nc.gpsimd.dma_start(out=bout_c[:], in_=bout_bcast)

        # per-point final tiles of K, Kx, Ky, f: [128, FT]
        kq = {}
        for name, d in (("K", K_d), ("Kx", Kx_d), ("Ky", Ky_d), ("f", f_d)):
            t = const.tile([128, FT], f32, tag=f"kq_{name}")
            nc.gpsimd.dma_start(
                out=t[:], in_=d[:].rearrange("(p j) -> p j", p=128)
            )
            kq[name] = t

        # ---- layer 0 input: deinterleave xy into feature-major rows ----
        # xyT rows 0,32 = x,y of chunk A; rows 64,96 = chunk B (32-aligned)
        xyT = sb.tile([128, F], mm_dt, tag="xyT")
        nc.vector.tensor_copy(xyT[:], zF[:])
        xyA = xystg[0:1, :].rearrange("p (b t) -> p b t", t=2)
        xyB = xystg[H : H + 1, :].rearrange("p (b t) -> p b t", t=2)
        nc.vector.tensor_copy(xyT[0:1, :], xyA[:, :, 0])
        nc.scalar.copy(xyT[32:33, :], xyA[:, :, 1])
        nc.vector.tensor_copy(xyT[H : H + 1, :], xyB[:, :, 0])
        nc.scalar.copy(xyT[96:97, :], xyB[:, :, 1])

        def mm_pass(out_ps, lhsT, rhs, start=True, stop=True):
            for j in range(NMM):
                s = slice(j * NH, (j + 1) * NH)
                nc.tensor.matmul(
                    out_ps[:, s], lhsT[:], rhs[:, s], start=start, stop=stop
                )

        # ---- layer 0 ----
        h = sb.tile([128, F], mm_dt, tag="h")
        hsq = sb.tile([128, F], ew_dt, tag="hsq")
        dbar = sb.tile([128, F], ew_dt, tag="dbar")
        gx = sb.tile([128, F], mm_dt, tag="gx")
        gy = sb.tile([128, F], mm_dt, tag="gy")
        m2 = sb.tile([128, F], ew_dt, tag="m2")
        lap0 = sb.tile([128, F], mm_dt, tag="dd")

        zp = ps.tile([128, F], f32, tag="zp")
        mm_pass(zp, WT[0], xyT)
        nc.scalar.activation(h[:], zp[:], AF.Tanh, bias=bcol[0], scale=1.0)
        nc.scalar.activation(hsq[:], h[:], AF.Square)
        nc.vector.tensor_scalar_add(dbar[:], hsq[:], -1.0)          # h^2-1 = -d
        # gx stored with flipped sign: gx = dbar*W0x = -(d*W0x)
        nc.vector.tensor_scalar_mul(gx[:], dbar[:], w0c[:, 0:1])
        nc.vector.tensor_scalar_mul(gy[:], dbar[:], w0c[:, 1:2])
        nc.vector.scalar_tensor_tensor(
            m2[:], h[:], 2.0, dbar[:], OP.mult, OP.mult
        )  # 2 h dbar = -2 h d = s/q-part
        nc.vector.tensor_scalar_mul(lap0[:], m2[:], q0p1[:])        # s * q0
        lsrc = [lap0]

        # ---- layers 1..3 ----
        for k in range(1, L):
            zp = ps.tile([128, F], f32, tag="zp")
            zxp = ps.tile([128, F], f32, tag="zxp")
            zyp = ps.tile([128, F], f32, tag="zyp")
            zlp = ps.tile([128, F], f32, tag="zlp")
            mm_pass(zp, WT[k], h)
            mm_pass(zxp, WT[k], gx)
            mm_pass(zyp, WT[k], gy)
            for i, ls in enumerate(lsrc):
                mm_pass(zlp, WT[k], ls, start=(i == 0), stop=(i == len(lsrc) - 1))

            h = sb.tile([128, F], mm_dt, tag="h")
            hsq = sb.tile([128, F], ew_dt, tag="hsq")
            dbar = sb.tile([128, F], ew_dt, tag="dbar")
            m2 = sb.tile([128, F], ew_dt, tag="m2")
            nc.scalar.activation(h[:], zp[:], AF.Tanh, bias=bcol[k], scale=1.0)
            nc.scalar.activation(hsq[:], h[:], AF.Square)
            nc.vector.tensor_scalar_add(dbar[:], hsq[:], -1.0)
            nc.vector.scalar_tensor_tensor(
                m2[:], h[:], 2.0, dbar[:], OP.mult, OP.mult
            )  # = s (off critical lap path)

            zxsq = sb.tile([128, F], ew_dt, tag="zxsq")
            zysq = sb.tile([128, F], ew_dt, tag="zysq")
            nc.scalar.activation(zxsq[:], zxp[:], AF.Square)
            nc.scalar.activation(zysq[:], zyp[:], AF.Square)

            gx = sb.tile([128, F], mm_dt, tag="gx")
            gy = sb.tile([128, F], mm_dt, tag="gy")
            nc.vector.scalar_tensor_tensor(
                gx[:], zxp[:], -1.0, dbar[:], OP.mult, OP.mult
            )  # zx*d
            nc.vector.scalar_tensor_tensor(
                gy[:], zyp[:], -1.0, dbar[:], OP.mult, OP.mult
            )

            q = sb.tile([128, F], ew_dt, tag="q")
            dd = sb.tile([128, F], mm_dt, tag="dd")
            ee = sb.tile([128, F], mm_dt, tag="ee")
            nc.vector.tensor_add(q[:], zxsq[:], zysq[:])
            nc.vector.tensor_mul(dd[:], q[:], m2[:])               # s*q
            nc.vector.scalar_tensor_tensor(
                ee[:], zlp[:], -1.0, dbar[:], OP.mult, OP.mult
            )  # d*zl
            lsrc = [dd, ee]

        # ---- output reductions: two psum tiles so u/ux/uy post-processing
        # overlaps the lap-path tail ----
        rp1 = ps.tile([8, F], f32, tag="zp")
        srcs1 = [(0, h), (1, gx), (2, gy)]
        for j in range(NMM):
            s = slice(j * NH, (j + 1) * NH)
            for i, (q_, src) in enumerate(srcs1):
                nc.tensor.matmul(
                    rp1[:, s], wout_l[q_], src[:, s],
                    start=(i == 0), stop=(i == len(srcs1) - 1),
                )
        rp2 = ps.tile([8, F], f32, tag="zyp")
        for j in range(NMM):
            s = slice(j * NH, (j + 1) * NH)
            for i, ls in enumerate(lsrc):
                nc.tensor.matmul(
                    rp2[:, s], wout_l[0], ls[:, s],
                    start=(i == 0), stop=(i == len(lsrc) - 1),
                )
        red1 = sb.tile([6, F], f32, tag="red1")
        red2 = sb.tile([2, F], f32, tag="red2")
        nc.vector.tensor_copy(red1[:], rp1[0:6, :])
        nc.vector.tensor_copy(red2[:], rp2[0:2, :])

        # ---- reshape rows -> [128, FT] per-point tiles ----
        fin = {}
        for q_, name in enumerate(("u", "ux", "uy")):
            t = sb.tile([128, FT], f32, tag=f"fin_{name}")
            nc.sync.dma_start(out=t[0:H, :], in_=red1[2 * q_ : 2 * q_ + 1, :])
            nc.scalar.dma_start(
                out=t[H:128, :], in_=red1[2 * q_ + 1 : 2 * q_ + 2, :]
            )
            fin[name] = t
        t = sb.tile([128, FT], f32, tag="fin_S")
        nc.sync.dma_start(out=t[0:H, :], in_=red2[0:1, :])
        nc.scalar.dma_start(out=t[H:128, :], in_=red2[1:2, :])
        fin["S"] = t

        # ---- final assembly ----
        u_fin = sb.tile([128, FT], f32, tag="u_fin")
        nc.vector.tensor_scalar_add(u_fin[:], fin["u"][:], bout_c[:])
        nc.sync.dma_start(out=u_d[:].rearrange("(p j) -> p j", p=128), in_=u_fin[:])

        t1 = sb.tile([128, FT], f32, tag="t1")
        t2 = sb.tile([128, FT], f32, tag="t2")
        pde = sb.tile([128, FT], f32, tag="pde")
        nc.vector.tensor_mul(t1[:], kq["Kx"][:], fin["ux"][:])
        nc.vector.tensor_sub(t1[:], kq["f"][:], t1[:])   # f - Kx*uxf (flipped)
        nc.vector.tensor_mul(t2[:], kq["Ky"][:], fin["uy"][:])
        nc.vector.tensor_sub(t1[:], t1[:], t2[:])
        nc.vector.tensor_mul(t2[:], kq["K"][:], fin["S"][:])
        nc.vector.tensor_add(pde[:], t1[:], t2[:])
        nc.sync.dma_start(
            out=pde_d[:].rearrange("(p j) -> p j", p=128), in_=pde[:]
        )

    if not nc.is_finalized():
        nc.finalize()
    legalized = _legalize_sync_waits(nc.to_json_bytes())
    nc.to_json_bytes = lambda: legalized
    return nc


_NC = None


def _get_nc():
    global _NC
    if _NC is None:
        _patch_walrus_ldw_opt()
        _NC = build_nc()
    return _NC


def make_in_maps(inputs):
    """Shard full inputs into per-core input maps."""
    full = {k: np.asarray(v, dtype=np.float32) for k, v in inputs.items()}
    in_maps = []
    for c in range(NCORES):
        s = slice(c * BC, (c + 1) * BC)
        m = {
            "xy": full["xy"][s],
            "K": full["K"][s],
            "Kx": full["Kx"][s],
            "Ky": full["Ky"][s],
            "f": full["f"][s],
            "Wout": full["Wout"],
            "bout": full["bout"].reshape(1),
        }
        for i in range(L):
            m[f"W{i}"] = full[f"W{i}"]
            m[f"b{i}"] = full[f"b{i}"]
        in_maps.append(m)
    return in_maps


def run(inputs, trace=False, **kw):
    from concourse.bass_utils import run_bass_kernel_spmd

    nc = _get_nc()
    res = run_bass_kernel_spmd(
        nc, make_in_maps(inputs), list(range(NCORES)), trace=trace, **kw
    )
    u = np.concatenate([r["u"] for r in res.results])
    pde = np.concatenate([r["pde"] for r in res.results])
    return (u, pde), res


def kernel(**inputs):
    (u, pde), _ = run(inputs)
    return u, pde



# revision 19
# speedup vs baseline: 1.1884x; 1.1884x over previous
"""Trainium2 Bass kernel for MembraneNet (PINN forward + analytic PDE residual).

Math (per collocation point): 4-layer tanh MLP u(x,y); PDE = K*(uxx+uyy)
+ Kx*ux + Ky*uy + f. Forward-mode propagation of (h, gx, gy, lap) per layer,
O(H^2)/point. Batch sharded 8 ways (2048 points/core); on each core points
sit in two 64-feature chunks on SBUF partitions 0-63/64-127 with
block-diagonal weights, 1024 columns per stream.

Design (v2, ~49us vs 58-65us v1):
- bf16 streams everywhere, f32 PSUM. DVE 2x modes on SBUF-only ops.
- Sign-carried streams remove the d=1-h^2 materialization: the gx/gy carrier
  alternates sign per layer under c' = (hsq-1).(W c) (absorbed by the
  reduction weights); the lap carrier stays lam = -lap via negated weights:
  p = (-W)@lam + (-I)@t on the PE (PSUM accumulation), lam' = (hsq-1).p.
- t = 2h(zx^2+zy^2) from one Act Square over the adjacent [ztx|zty] PSUM
  slab with scale=sqrt(2).
- Software pipeline with 1-layer skew: stage A(k) (h/gxy chain, independent
  of the lap path) emitted as A1 A2 B1 A3 B2 B3 so B(k) (lap tail) fills
  queue gaps. Separate zxy/pp/zp PSUM tiles avoid cross-path WAR stalls.
- All weights/constants host-preformatted into packed DMAs; L0 via a
  zero-padded lhsT on rows 0-3 (xy rows). PE warmup matmuls while input
  DMAs land. Per-quantity output reductions land in freed PSUM regions so
  the u/ux/uy epilogue hides under the lap tail; only S is tail-serial.
"""

import sys

sys.path.insert(0, "/opt/trn_rl_repo")

import numpy as np
from contextlib import ExitStack

import concourse.bass as bass
import concourse.mybir as mybir
import concourse.tile as tile

B = 16384
H = 64
L = 4
NCORES = 8
BC = B // NCORES          # 2048 points per core
F = BC // 2               # 1024 columns (2 chunks of 1024 points on partitions)
FT = BC // 128            # 16: free dim of final per-point [128, FT] tiles

f32 = mybir.dt.float32
bf16 = mybir.dt.bfloat16
AF = mybir.ActivationFunctionType
OP = mybir.AluOpType

SQRT2 = float(np.sqrt(2.0))

# wpack bf16 column layout
WP_WT = [None, 0, 128, 256]        # WT[k] blockdiag(Wk.T,Wk.T), k=1..3
WP_NWT = [None, 384, 512, 640]     # -WT[k]
WP_NEGI = 768                      # -I128
WP_WL = 896                        # wl32 reduction lhsT [128,32]
WP_W0T4 = 928                      # L0 lhsT [128,128], rows 0-3 live
WP_COLS = 1056

# cpack f32 column layout: b0..b3, -2*q0, w0x, w0y, bout
CP_B = [0, 1, 2, 3]
CP_M2Q0 = 4
CP_W0X = 5
CP_W0Y = 6
CP_BOUT = 7
CP_COLS = 8

WARMUP_MM = 0


def _legalize_sync_waits(bj: bytes) -> bytes:
    """The walrus in this container accepts at most ONE on_wait per
    instruction, but Tile emits several. Move excess waits into standalone
    EventSemaphore instructions right before the owner (same engine, so the
    sequencer executes them first) — the exact encoding raw-bass wait_ge uses.
    """
    import json

    m = json.loads(bj)
    n = 0
    for fn in m.get("functions", []):
        for blk in fn.get("blocks", []):
            out = []
            for ins in blk.get("instructions", []):
                si = ins.get("sync_info") or {}
                waits = si.get("on_wait") or []
                if len(waits) > 1:
                    for w in waits[:-1]:
                        n += 1
                        out.append(
                            {
                                "name": f"lsw_{n}",
                                "opcode": "EventSemaphore",
                                "engine": ins["engine"],
                                "ins": [],
                                "outs": [],
                                "debug": ins.get("debug", 0),
                                "sync_info": {"on_update": [], "on_wait": [w]},
                            }
                        )
                    si["on_wait"] = waits[-1:]
                out.append(ins)
            blk["instructions"] = out
    return json.dumps(m).encode()


def build_nc():
    nc = bass.Bass()

    # ---- I/O (everything preformatted on host) ----
    xyr_d = nc.dram_tensor("xyr", [4, F], bf16, kind="ExternalInput")
    wpack_d = nc.dram_tensor("wpack", [128, WP_COLS], bf16, kind="ExternalInput")
    cpack_d = nc.dram_tensor("cpack", [128, CP_COLS], f32, kind="ExternalInput")
    kq_d = nc.dram_tensor("kq", [128, 4 * FT], f32, kind="ExternalInput")
    u_d = nc.dram_tensor("u", [BC], f32, kind="ExternalOutput")
    pde_d = nc.dram_tensor("pde", [BC], f32, kind="ExternalOutput")

    with tile.TileContext(nc) as tc, ExitStack() as ctx:
        const = ctx.enter_context(tc.tile_pool(name="const", bufs=1))
        sb = ctx.enter_context(tc.tile_pool(name="sb", bufs=3))
        ps = ctx.enter_context(tc.tile_pool(name="ps", bufs=1, space="PSUM"))

        # ---- input DMAs, spread across queues ----
        wpack = const.tile([128, WP_COLS], bf16, tag="wpack")
        nc.sync.dma_start(out=wpack[:], in_=wpack_d[:, :])
        cpack = const.tile([128, CP_COLS], f32, tag="cpack")
        nc.scalar.dma_start(out=cpack[:], in_=cpack_d[:, :])
        xypad = const.tile([128, F], bf16, tag="xypad")
        nc.vector.memset(xypad[:], 0.0)
        nc.gpsimd.dma_start(out=xypad[0:4, :], in_=xyr_d[:, :])
        kq = const.tile([128, 4 * FT], f32, tag="kq")
        nc.gpsimd.dma_start(out=kq[:], in_=kq_d[:, :])

        WT = [None] + [wpack[:, WP_WT[k] : WP_WT[k] + 128] for k in (1, 2, 3)]
        NWT = [None] + [wpack[:, WP_NWT[k] : WP_NWT[k] + 128] for k in (1, 2, 3)]
        NEGI = wpack[:, WP_NEGI : WP_NEGI + 128]
        WL = [wpack[:, WP_WL + 2 * q : WP_WL + 2 * q + 2] for q in range(4)]
        W0T = wpack[:, WP_W0T4 : WP_W0T4 + 128]
        bcol = [cpack[:, k : k + 1] for k in CP_B]
        m2q0 = cpack[:, CP_M2Q0 : CP_M2Q0 + 1]
        w0x = cpack[:, CP_W0X : CP_W0X + 1]
        w0y = cpack[:, CP_W0Y : CP_W0Y + 1]
        boutc = cpack[:, CP_BOUT : CP_BOUT + 1]

        # ---- ACT table warmup (hide the ~1.3us table load under DMA wait) ----
        wrm = const.tile([1, 1], f32, tag="wrm")
        nc.vector.memset(wrm[:], 0.0)
        nc.scalar.activation(wrm[:], wrm[:], AF.Tanh)

        # ---- PE warmup: junk matmuls into the zxyp PSUM region ----
        jnk = const.tile([128, 512], bf16, tag="jnk")
        nc.vector.memset(jnk[:], 0.0)
        jp = ps.tile([128, 512], f32, tag="zxyp")
        for _ in range(WARMUP_MM):
            nc.tensor.matmul(jp[:], jnk[:, 0:128], jnk[:], start=True, stop=True)

        NH = 512

        def mm(out, lhsT, rhs, start=True, stop=True):
            for j in range(0, out.shape[-1], NH):
                nc.tensor.matmul(
                    out[:, j : j + NH], lhsT, rhs[:, j : j + NH],
                    start=start, stop=stop,
                )

        # ---- layer 0 ----
        zp = ps.tile([128, F], f32, tag="zp")
        mm(zp[:], W0T, xypad[:])

        h = sb.tile([128, F], bf16, tag="h")
        hsq = sb.tile([128, F], bf16, tag="hsq")
        nc.scalar.activation(h[:], zp[:], AF.Tanh, bias=bcol[0], scale=1.0)
        nc.scalar.activation(hsq[:], h[:], AF.Square)
        dbar = sb.tile([128, F], bf16, tag="dbar")
        nc.vector.tensor_scalar_add(dbar[:], hsq[:], -1.0)
        gfl = sb.tile([128, 3 * F], bf16, tag="gfl")
        nc.vector.tensor_scalar_mul(gfl[:, 0:F], dbar[:], w0x)       # c = -g0
        nc.vector.tensor_scalar_mul(gfl[:, F : 2 * F], dbar[:], w0y)
        nc.vector.tensor_mul(m[:], h[:], dbar[:])                    # h*(-d)
        nc.vector.tensor_scalar_mul(gfl[:, 2 * F : 3 * F], m[:], m2q0)

        # ---- layers 1..3 ----
        for k in range(1, L):
            zxyp = ps.tile([128, 3 * F], f32, tag="zxyp")
            mm(zxyp[:, 0:F], WT[k], gfl[:, 0:F])
            mm(zxyp[:, F : 2 * F], WT[k], gfl[:, F : 2 * F])
            zp = ps.tile([128, F], f32, tag="zp")
            mm(zp[:], WT[k], h[:])

            hN = sb.tile([128, F], bf16, tag="h")
            hsqN = sb.tile([128, F], bf16, tag="hsq")
            nc.scalar.activation(hN[:], zp[:], AF.Tanh, bias=bcol[k], scale=1.0)
            nc.scalar.activation(hsqN[:], hN[:], AF.Square)

            # q2 = 2*(zx^2+zy^2) via one Square over the [ztx|zty] slab
            sqs = sb.tile([128, 2 * F], bf16, tag="sqs")
            nc.scalar.activation(sqs[:], zxyp[:, 0 : 2 * F], AF.Square, scale=SQRT2)
            q2 = sb.tile([128, F], bf16, tag="q2")
            nc.vector.tensor_add(q2[:], sqs[:, 0:F], sqs[:, F : 2 * F])
            t = sb.tile([128, F], bf16, tag="t")
            nc.vector.tensor_mul(t[:], hN[:], q2[:])

            # p = (-W)@lam + (-I)@t  (PSUM accumulation)
            mm(zxyp[:, 2 * F : 3 * F], NWT[k], gfl[:, 2 * F : 3 * F],
               start=True, stop=False)
            mm(zxyp[:, 2 * F : 3 * F], NEGI, t[:], start=False, stop=True)

            gflN = sb.tile([128, 3 * F], bf16, tag="gfl")
            # gxy: (hsq-1) (.) [ztx|zty] — hsq broadcast over the 2 thirds
            hsq_rep = bass.AP(
                tensor=hsqN[:].tensor,
                offset=hsqN[:].offset,
                ap=[[hsqN[:].ap[0][0], 128], [0, 2], [1, F]],
            )
            nc.vector.scalar_tensor_tensor(
                gflN[:, 0 : 2 * F].rearrange("p (a b) -> p a b", a=2),
                hsq_rep, -1.0, zxyp[:, 0 : 2 * F].rearrange("p (a b) -> p a b", a=2),
                OP.add, OP.mult,
            )
            # lam' = (hsq-1) (.) p
            nc.vector.scalar_tensor_tensor(
                gflN[:, 2 * F : 3 * F],
                hsqN[:], -1.0, zxyp[:, 2 * F : 3 * F],
                OP.add, OP.mult,
            )
            h, hsq, gfl = hN, hsqN, gflN

        # ---- output reductions: independent per-quantity groups that land
        # in freed PSUM regions, so u/ux/uy hide under the lap tail ----
        redu = sb.tile([2, F], f32, tag="redu")
        redx = sb.tile([2, F], f32, tag="redx")
        redy = sb.tile([2, F], f32, tag="redy")
        reds = sb.tile([2, F], f32, tag="reds")
        fin = sb.tile([128, 4 * FT], f32, tag="fin")
        dmaq = [nc.sync, nc.scalar]

        ru = ps.tile([2, F], f32, tag="zp")
        for b in range(NB):
            mm(ru[:, BS(b)], WL[0], h[:, BS(b)])
        nc.scalar.copy(redu[:], ru[:])
        for c in range(2):
            dmaq[c % 2].dma_start(
                out=fin[64 * c : 64 * (c + 1), 0:FT], in_=redu[c : c + 1, :]
            )
        u_fin = sb.tile([128, FT], f32, tag="u_fin")
        nc.vector.tensor_scalar_add(u_fin[:], fin[:, 0:FT], boutc)
        nc.sync.dma_start(
            out=u_d[:].rearrange("(p j) -> p j", p=128), in_=u_fin[:]
        )

        rxy = ps.tile([2, 2 * F], f32, tag="zxy")
        for b in range(NB):
            mm(rxy[:, BS(b)], WL[1], gfl[:, b * FB : b * FB + FB])
            mm(rxy[:, F + b * FB : F + b * FB + FB], WL[2],
               gfl[:, F + b * FB : F + b * FB + FB])
        nc.vector.tensor_copy(redx[:], rxy[:, 0:F])
        nc.scalar.copy(redy[:], rxy[:, F : 2 * F])
        for q, rr in ((1, redx), (2, redy)):
            for c in range(2):
                dmaq[c % 2].dma_start(
                    out=fin[64 * c : 64 * (c + 1), FT * q : FT * (q + 1)],
                    in_=rr[c : c + 1, :],
                )
        # pde1 = f + Kx*ux + Ky*uy  (hidden under the lap tail)
        prods = sb.tile([128, 2 * FT], f32, tag="prods")
        nc.vector.tensor_mul(
            prods[:], kq[:, FT : 3 * FT], fin[:, FT : 3 * FT]
        )
        pde1 = sb.tile([128, FT], f32, tag="s1")
        nc.vector.tensor_add(pde1[:], prods[:, 0:FT], prods[:, FT : 2 * FT])
        nc.vector.tensor_add(pde1[:], pde1[:], kq[:, 0:FT])

        # S: the only tail-serial quantity
        rs = ps.tile([2, F], f32, tag="pp")
        for b in range(NB):
            mm(rs[:, BS(b)], WL[3], gfl[:, 2 * F + b * FB : 2 * F + b * FB + FB])
        nc.vector.tensor_copy(reds[:, 0 : F // 2], rs[:, 0 : F // 2])
        nc.scalar.copy(reds[:, F // 2 : F], rs[:, F // 2 : F])
        for c in range(2):
            dmaq[c % 2].dma_start(
                out=fin[64 * c : 64 * (c + 1), 3 * FT : 4 * FT],
                in_=reds[c : c + 1, :],
            )
        prodS = sb.tile([128, FT], f32, tag="prodS")
        nc.vector.tensor_mul(prodS[:], kq[:, 3 * FT : 4 * FT], fin[:, 3 * FT : 4 * FT])
        pde = sb.tile([128, FT], f32, tag="pde")
        nc.vector.tensor_add(pde[:], pde1[:], prodS[:])
        nc.sync.dma_start(
            out=pde_d[:].rearrange("(p j) -> p j", p=128), in_=pde[:]
        )

    if not nc.is_finalized():
        nc.finalize()
    legalized = _legalize_sync_waits(nc.to_json_bytes())
    nc.to_json_bytes = lambda: legalized
    return nc


_NC = None


def _get_nc():
    global _NC
    if _NC is None:
        # ldw-opt (LDWEIGHTS dedup) rejects bf16 FWL loads; FWL makes
        # reloads cheap (~30ns) so dedup is not worth it here.
        _NC = build_nc()
    return _NC


def _host_prep(full):
    """Build the shared (weight/const) arrays once."""
    import ml_dtypes

    b16 = ml_dtypes.bfloat16
    W = [full[f"W{i}"] for i in range(L)]
    bvec = [full[f"b{i}"] for i in range(L)]
    Wout = full["Wout"]
    bout = float(full["bout"])

    wpack = np.zeros((128, WP_COLS), np.float32)
    for k in (1, 2, 3):
        wt = W[k].T  # [in, out] = Wk.T so lhsT.T @ rhs = Wk @ rhs
        wpack[0:H, WP_WT[k] : WP_WT[k] + H] = wt
        wpack[H:128, WP_WT[k] + H : WP_WT[k] + 128] = wt
        wpack[0:H, WP_NWT[k] : WP_NWT[k] + H] = -wt
        wpack[H:128, WP_NWT[k] + H : WP_NWT[k] + 128] = -wt
    wpack[:, WP_NEGI : WP_NEGI + 128] = -np.eye(128, dtype=np.float32)
    wpack[0, WP_W0T4 : WP_W0T4 + H] = W[0][:, 0]
    wpack[1, WP_W0T4 : WP_W0T4 + H] = W[0][:, 1]
    wpack[2, WP_W0T4 + H : WP_W0T4 + 128] = W[0][:, 0]
    wpack[3, WP_W0T4 + H : WP_W0T4 + 128] = W[0][:, 1]
    # reduction lhsT: quantity q -> col 2q = chunkA, col 2q+1 = chunkB
    sgn = [1.0, 1.0, 1.0, -1.0]  # u, ux, uy, S(lam carries -lap)
    for q in range(4):
        wpack[0:H, WP_WL + 2 * q] = sgn[q] * Wout
        wpack[H:128, WP_WL + 2 * q + 1] = sgn[q] * Wout
    wpack = wpack.astype(b16)

    cpack = np.zeros((128, CP_COLS), np.float32)
    for k in range(L):
        cpack[0:H, CP_B[k]] = bvec[k]
        cpack[H:128, CP_B[k]] = bvec[k]
    q0 = W[0][:, 0] ** 2 + W[0][:, 1] ** 2
    cpack[0:H, CP_M2Q0] = -2.0 * q0
    cpack[H:128, CP_M2Q0] = -2.0 * q0
    cpack[0:H, CP_W0X] = W[0][:, 0]
    cpack[H:128, CP_W0X] = W[0][:, 0]
    cpack[0:H, CP_W0Y] = W[0][:, 1]
    cpack[H:128, CP_W0Y] = W[0][:, 1]
    cpack[:, CP_BOUT] = bout

    return wpack, cpack, b16


def make_in_maps(inputs):
    full = {k: np.asarray(v, dtype=np.float32) for k, v in inputs.items()}
    wpack, cpack, b16 = _host_prep(full)
    in_maps = []
    for c in range(NCORES):
        s = slice(c * BC, (c + 1) * BC)
        xy = full["xy"][s]
        xyr = np.stack(
            [xy[0:F, 0], xy[0:F, 1], xy[F:BC, 0], xy[F:BC, 1]]
        ).astype(b16)
        kqa = np.concatenate(
            [full[n][s].reshape(128, FT) for n in ("f", "Kx", "Ky", "K")],
            axis=1,
        )
        in_maps.append(
            {
                "xyr": xyr,
                "wpack": wpack,
                "cpack": cpack,
                "kq": kqa,
            }
        )
    return in_maps


def run(inputs, trace=False, **kw):
    from concourse.bass_utils import run_bass_kernel_spmd

    nc = _get_nc()
    res = run_bass_kernel_spmd(
        nc, make_in_maps(inputs), list(range(NCORES)), trace=trace, **kw
    )
    u = np.concatenate([r["u"] for r in res.results])
    pde = np.concatenate([r["pde"] for r in res.results])
    return (u, pde), res


def kernel(**inputs):
    (u, pde), _ = run(inputs)
    return u, pde
